# revision 8
# baseline (speedup 1.0000x reference)
"""Cosformer attention Trainium2 kernel (fp16 compute / bf16 post-z edition).

Shards batch*heads across 8 NeuronCores: core c handles batch c//4 and
heads 4*(c%4) .. 4*(c%4)+4 (a 256-wide slice of the embedding). Each core:
  - projects q/k/v from its batch's query slice (fp16 matmuls, fp32 PSUM),
  - applies RoPE + relu; q' is pre-scaled by the cosformer sin/cos weights
    into two resident copies qs = q'*s_l and qc = q'*c_l (f-major),
  - computes the per-head cosformer kv summary (2D x D) + k-sums over all L,
  - pass 2: ab = qs@kv_top + qc@kv_bot accumulates numerator AND denominator
    in one PSUM group; z = 1/max(den,eps); attn scaled on gpsimd,
  - multiplies by its slice of Wo, producing a partial (L, E) output in bf16.
Host sums the 4 partials per batch in fp32. No cross-device communication.

Precision: the cosformer denominator crosses zero (min |den| ~ 2.8 vs median
~3800), so any 0.4%-level (bf16) noise upstream of z is catastrophic; fp16
(10-bit mantissa) matches the fp32r baseline's stability at half the DMA and
with the DVE 2x 16-bit mode. Post-z tensors (attn/out) overflow fp16's range
(z can be 1e6), so they are bf16. PSUM accumulation is fp32 throughout.
"""

import os
import sys

if "/opt/trn_rl_repo" not in sys.path:
    sys.path.insert(0, "/opt/trn_rl_repo")

from contextlib import ExitStack

import numpy as np
import ml_dtypes

import concourse.bass as bass
import concourse.bacc as bacc
import concourse.mybir as mybir
import concourse.tile as tile

F32 = mybir.dt.float32
BF16 = mybir.dt.bfloat16
F16 = mybir.dt.float16
BF = ml_dtypes.bfloat16
EPS = 1e-6

L_FULL, N_BATCH, E, H, D = 4096, 2, 1024, 16, 64
N_CORES = 8
HEADS_PER_CORE = 4          # 2 pairs
F_LOC = HEADS_PER_CORE * D  # 256


def build_program(LT=32):
    """Build the single-core SPMD Bass program. LT = number of 128-row L tiles."""
    L = LT * 128
    CH = 512                    # L columns per q-projection chunk
    NCH = L // CH
    LT_PER_CH = CH // 128

    nc = bacc.Bacc("TRN2", target_bir_lowering=False, debug=False)

    qbT_d = nc.dram_tensor("qbT", [E, L], F16, kind="ExternalInput").ap()
    wqT_d = nc.dram_tensor("wqT", [E, F_LOC], F16, kind="ExternalInput").ap()
    wkvT_d = nc.dram_tensor("wkvT", [E, 2 * F_LOC], F16, kind="ExternalInput").ap()
    wo_d = nc.dram_tensor("wo_rhs", [F_LOC, E], BF16, kind="ExternalInput").ap()
    cosT_d = nc.dram_tensor("cosT_rep", [128, L], F16, kind="ExternalInput").ap()
    sinT_d = nc.dram_tensor("sinT_rep", [128, L], F16, kind="ExternalInput").ap()
    sT_d = nc.dram_tensor("sT_rep", [128, L], F16, kind="ExternalInput").ap()
    cT_d = nc.dram_tensor("cT_rep", [128, L], F16, kind="ExternalInput").ap()
    cosL_d = nc.dram_tensor("cosL", [128, LT * D], F16, kind="ExternalInput").ap()
    sinmL_d = nc.dram_tensor("sinmL", [128, LT * D], F16, kind="ExternalInput").ap()
    sc_d = nc.dram_tensor("sc_t", [128, LT * 2], F32, kind="ExternalInput").ap()
    scr_d = nc.dram_tensor("sc_tr", [128, LT * 2], F16, kind="ExternalInput").ap()
    p2_d = nc.dram_tensor("p2_rot", [128, 128], F16, kind="ExternalInput").ap()
    id_d = nc.dram_tensor("ident_r", [128, 128], BF16, kind="ExternalInput").ap()
    out_d = nc.dram_tensor("out_partial", [L, E], BF16, kind="ExternalOutput").ap()

    with tile.TileContext(nc) as tc, ExitStack() as ctx:
        consts = ctx.enter_context(tc.tile_pool(name="consts", bufs=1))

        # ---- resident constants / inputs ----
        qc_sb = consts.tile([128, 8 * L], F16, name="qc_sb")       # whole input
        wq_sb = consts.tile([128, 8 * F_LOC], F16, name="wq_sb")
        wkv_sb = consts.tile([128, 8 * 2 * F_LOC], F16, name="wkv_sb")
        wo_sb = consts.tile([128, 2 * E], BF16, name="wo_sb")
        cosT_sb = consts.tile([128, L], F16, name="cosT_sb")
        sinT_sb = consts.tile([128, L], F16, name="sinT_sb")
        sT_sb = consts.tile([128, L], F16, name="sT_sb")
        cT_sb = consts.tile([128, L], F16, name="cT_sb")
        cosL_sb = consts.tile([128, LT * D], F16, name="cosL_sb")
        sinmL_sb = consts.tile([128, LT * D], F16, name="sinmL_sb")
        sc_sb = consts.tile([128, LT * 2], F32, name="sc_sb")
        scr_sb = consts.tile([128, LT * 2], F16, name="scr_sb")
        p2_sb = consts.tile([128, 128], F16, name="p2_sb")
        ident = consts.tile([128, 128], BF16, name="ident")

        qc3 = qc_sb[:].rearrange("p (e l) -> p e l", e=8)
        wq3 = wq_sb[:].rearrange("p (e f) -> p e f", e=8)
        wkv3 = wkv_sb[:].rearrange("p (e f) -> p e f", e=8)
        wo3 = wo_sb[:].rearrange("p (k j) -> p k j", k=2)
        qd3 = qbT_d.rearrange("(e p) l -> p e l", p=128)
        wqd3 = wqT_d.rearrange("(e p) f -> p e f", p=128)
        wkvd3 = wkvT_d.rearrange("(e p) f -> p e f", p=128)

        # resident cosformer-scaled q^T (f-major, per pair) and kv summaries
        qsT_sb = consts.tile([128, 2 * L], F16, name="qsT_sb")
        qcT_sb = consts.tile([128, 2 * L], F16, name="qcT_sb")
        qs3 = qsT_sb[:].rearrange("p (r l) -> p r l", r=2)
        qx3 = qcT_sb[:].rearrange("p (r l) -> p r l", r=2)
        kvtop_sb = [consts.tile([128, 130], F16, name=f"kvtop{pr}") for pr in range(2)]
        kvbot_sb = [consts.tile([128, 130], F16, name=f"kvbot{pr}") for pr in range(2)]

        # ---- DMA issue order: critical inputs first, spread across queues ----
        QQ = 4                       # input quarters (1024 L-cols each)
        LQ = L // QQ
        for e in range(8):
            nc.sync.dma_start(qc3[:, e:e + 1, 0:LQ], qd3[:, e:e + 1, 0:LQ])
            nc.scalar.dma_start(wq3[:, e:e + 1, :], wqd3[:, e:e + 1, :])
            nc.gpsimd.dma_start(wkv3[:, e:e + 1, :], wkvd3[:, e:e + 1, :])
        nc.gpsimd.dma_start(cosL_sb[:], cosL_d[:])
        nc.gpsimd.dma_start(sinmL_sb[:], sinmL_d[:])
        nc.gpsimd.dma_start(sc_sb[:], sc_d[:])
        nc.gpsimd.dma_start(scr_sb[:], scr_d[:])
        nc.scalar.dma_start(p2_sb[:], p2_d[:])
        nc.scalar.dma_start(cosT_sb[:], cosT_d[:])
        nc.scalar.dma_start(sinT_sb[:], sinT_d[:])
        nc.scalar.dma_start(sT_sb[:], sT_d[:])
        nc.scalar.dma_start(cT_sb[:], cT_d[:])
        nc.gpsimd.dma_start(ident[:], id_d[:])
        for pr in range(2):
            nc.gpsimd.memset(kvtop_sb[pr][:], 0.0)
            nc.gpsimd.memset(kvbot_sb[pr][:], 0.0)
        for qq in range(1, QQ):
            for e in range(8):
                nc.sync.dma_start(qc3[:, e:e + 1, qq * LQ:(qq + 1) * LQ],
                                  qd3[:, e:e + 1, qq * LQ:(qq + 1) * LQ])
        nc.sync.dma_start(
            wo_sb[:].rearrange("p (k j) -> p k j", k=2),
            wo_d.rearrange("(k p) j -> p k j", p=128),
        )

        # ================= PASS 1 =================
        with ExitStack() as p1:
            proj_ps = p1.enter_context(tc.tile_pool(name="proj_ps", bufs=3, space="PSUM"))
            q_ps_pool = p1.enter_context(tc.tile_pool(name="q_ps", bufs=3, space="PSUM"))
            kv_ps_pool = p1.enter_context(tc.tile_pool(name="kv_ps", bufs=1, space="PSUM"))
            wk1 = p1.enter_context(tc.tile_pool(name="wk1", bufs=3))

            kv_ps = [kv_ps_pool.tile([128, 258], F32, name=f"kv_ps{pr}") for pr in range(2)]

            for ch in range(NCH):
                csl = slice(ch * CH, (ch + 1) * CH)

                # ---- q: f-major projection + relu + RoPE + s/c scale (per pair) ----
                for pr in range(2):
                    qT_ps = q_ps_pool.tile([128, CH], F32, name="qT_ps", tag="q512")
                    for e in range(8):
                        nc.tensor.matmul(
                            qT_ps[:],
                            wq3[:, e, pr * 128:(pr + 1) * 128],
                            qc3[:, e, csl],
                            start=(e == 0), stop=(e == 7),
                        )
                    qTr = wk1.tile([128, CH], F16, name="qTr", tag="qTr")
                    nc.scalar.activation(qTr[:], qT_ps[:], mybir.ActivationFunctionType.Relu)
                    rot_ps = q_ps_pool.tile([128, CH], F32, name="rot_ps", tag="q512")
                    nc.tensor.matmul(
                        rot_ps[:], p2_sb[:], qTr[:],
                        start=True, stop=True,
                    )
                    tq = wk1.tile([128, CH], F16, name="tq", tag="tq")
                    nc.vector.tensor_tensor(
                        out=tq[:], in0=qTr[:], in1=cosT_sb[:, csl], op=mybir.AluOpType.mult)
                    uq = wk1.tile([128, CH], F16, name="uq", tag="uq")
                    nc.vector.tensor_tensor(
                        out=uq[:], in0=rot_ps[:], in1=sinT_sb[:, csl], op=mybir.AluOpType.mult)
                    qq_t = wk1.tile([128, CH], F16, name="qq_t", tag="qq_t")
                    nc.vector.tensor_tensor(
                        out=qq_t[:], in0=tq[:], in1=uq[:], op=mybir.AluOpType.add)
                    nc.vector.tensor_tensor(
                        out=qs3[:, pr, csl], in0=qq_t[:], in1=sT_sb[:, csl],
                        op=mybir.AluOpType.mult)
                    nc.vector.tensor_tensor(
                        out=qx3[:, pr, csl], in0=qq_t[:], in1=cT_sb[:, csl],
                        op=mybir.AluOpType.mult)

                # ---- k, v: L-major projections, per L-tile ----
                for lt in range(LT_PER_CH):
                    t = ch * LT_PER_CH + lt
                    lsl = slice(t * 128, (t + 1) * 128)
                    kv_proj_ps = proj_ps.tile([128, 2 * F_LOC], F32, name="kv_proj_ps", tag="proj")
                    k_ps = kv_proj_ps[:, 0:F_LOC]
                    for e in range(8):
                        nc.tensor.matmul(kv_proj_ps[:], qc3[:, e, lsl], wkv3[:, e, :],
                                         start=(e == 0), stop=(e == 7))

                    # k: relu then RoPE (L-major; swap via reversed AP, signed sin)
                    k_sb = wk1.tile([128, F_LOC], F16, name="k_sb", tag="k_sb")
                    nc.scalar.activation(k_sb[:], k_ps, mybir.ActivationFunctionType.Relu)
                    dsl = slice(t * D, (t + 1) * D)
                    cosL_t = cosL_sb[:, dsl].rearrange("p (a j) -> p a j", a=2) \
                        .unsqueeze(1).broadcast_to([128, 4, 2, 32])
                    sinm_t = sinmL_sb[:, dsl].rearrange("p (a j) -> p a j", a=2) \
                        .unsqueeze(1).broadcast_to([128, 4, 2, 32])
                    kt1 = wk1.tile([128, F_LOC], F16, name="kt1", tag="kt1")
                    nc.vector.tensor_tensor(
                        out=kt1[:].rearrange("p (h a j) -> p h a j", h=4, a=2),
                        in0=k_sb[:].rearrange("p (h a j) -> p h a j", h=4, a=2),
                        in1=cosL_t, op=mybir.AluOpType.mult)
                    kt2 = wk1.tile([128, F_LOC], F16, name="kt2", tag="kt2")
                    nc.vector.tensor_tensor(
                        out=kt2[:].rearrange("p (h a j) -> p h a j", h=4, a=2),
                        in0=k_sb[:].rearrange("p (h a j) -> p h a j", h=4, a=2)[:, :, ::-1, :],
                        in1=sinm_t, op=mybir.AluOpType.mult)
                    nc.vector.tensor_tensor(out=kt1[:], in0=kt1[:], in1=kt2[:],
                                            op=mybir.AluOpType.add)

                    # v: evict raw (Act), scale by s/c on gpsimd in SBUF
                    v_raw = wk1.tile([128, F_LOC], F16, name="v_raw", tag="v_raw")
                    nc.scalar.activation(v_raw[:], kv_proj_ps[:, F_LOC:2 * F_LOC],
                                         mybir.ActivationFunctionType.Copy)
                    vsc = wk1.tile([128, 520], F16, name="vsc", tag="vsc")
                    s_col = sc_sb[:, 2 * t:2 * t + 1]
                    c_col = sc_sb[:, 2 * t + 1:2 * t + 2]
                    for pr in range(2):
                        base = pr * 260
                        vp = v_raw[:, pr * 128:(pr + 1) * 128]
                        nc.gpsimd.tensor_scalar_mul(vsc[:, base:base + 128], vp, s_col)
                        nc.gpsimd.tensor_scalar_mul(vsc[:, base + 128:base + 256], vp, c_col)
                        nc.gpsimd.tensor_copy(vsc[:, base + 256:base + 258],
                                              scr_sb[:, 2 * t:2 * t + 2])

                    # kv accumulation (single group per pair incl k-sums)
                    for pr in range(2):
                        psl = slice(pr * 128, (pr + 1) * 128)
                        nc.tensor.matmul(
                            kv_ps[pr][:],
                            kt1[:, psl],
                            vsc[:, pr * 260:pr * 260 + 258],
                            start=(t == 0), stop=(t == LT - 1),
                        )

            # ---- kv eviction / per-head rearrangement ----
            # kv_ps[pr]: rows 0:64 = head A (d), 64:128 = head B;
            # cols 0:128 kv_top (A cols 0:64, B cols 64:128), 128:256 kv_bot, 256:258 ksum t/b.
            # kvtop_sb[pr] block-diagonal (128, 130): rows 0:64 (head A d) hold A's
            # [kvtop|kstop] in cols 0:65; rows 64:128 hold B's in cols 65:130.
            CP = mybir.ActivationFunctionType.Copy
            for pr in range(2):
                nc.vector.tensor_copy(kvtop_sb[pr][0:64, 0:64], kv_ps[pr][0:64, 0:64])
                nc.vector.tensor_copy(kvtop_sb[pr][0:64, 64:65], kv_ps[pr][0:64, 256:257])
                nc.vector.tensor_copy(kvtop_sb[pr][64:128, 65:129], kv_ps[pr][64:128, 64:128])
                nc.vector.tensor_copy(kvtop_sb[pr][64:128, 129:130], kv_ps[pr][64:128, 256:257])
                nc.scalar.activation(kvbot_sb[pr][0:64, 0:64], kv_ps[pr][0:64, 128:192], CP)
                nc.scalar.activation(kvbot_sb[pr][0:64, 64:65], kv_ps[pr][0:64, 257:258], CP)
                nc.scalar.activation(kvbot_sb[pr][64:128, 65:129], kv_ps[pr][64:128, 192:256], CP)
                nc.scalar.activation(kvbot_sb[pr][64:128, 129:130], kv_ps[pr][64:128, 257:258], CP)

        # ================= PASS 2 =================
        with ExitStack() as p2:
            ab_ps_pool = p2.enter_context(tc.tile_pool(name="ab_ps", bufs=3, space="PSUM"))
            tp_ps_pool = p2.enter_context(tc.tile_pool(name="tp_ps", bufs=2, space="PSUM"))
            op_ps_pool = p2.enter_context(tc.tile_pool(name="op_ps", bufs=3, space="PSUM"))
            wk2 = p2.enter_context(tc.tile_pool(name="wk2", bufs=3))

            for t in range(LT):
                l0 = t * 128
                lsl = slice(l0, l0 + 128)
                # ab holds both pairs: cols 0:130 = pr0 [Anum|Aden|Bnum|Bden], 130:260 = pr1.
                # Groups are strictly sequential per region (start..stop closed
                # before the next opens) -- interleaved open groups corrupt PSUM.
                ab_ps = ab_ps_pool.tile([128, 260], F32, name="ab_ps", tag="ab")
                for pr in range(2):
                    asl = slice(pr * 130, (pr + 1) * 130)
                    nc.tensor.matmul(
                        ab_ps[:, asl],
                        qs3[:, pr:pr + 1, lsl].squeeze(1),
                        kvtop_sb[pr][:],
                        start=True, stop=False,
                    )
                    nc.tensor.matmul(
                        ab_ps[:, asl],
                        qx3[:, pr:pr + 1, lsl].squeeze(1),
                        kvbot_sb[pr][:],
                        start=False, stop=True,
                    )
                ab4 = ab_ps[:].rearrange("p (h x) -> p h x", h=4)
                z = wk2.tile([128, 4], F32, name="z", tag="z")
                nc.vector.tensor_scalar_max(z[:], ab4[:, :, 64], EPS)
                nc.vector.reciprocal(z[:], z[:])
                attn = wk2.tile([128, 256], BF16, name="attn", tag="attn")
                nc.vector.tensor_tensor(
                    out=attn[:].rearrange("p (h j) -> p h j", h=4),
                    in0=ab4[:, :, 0:64],
                    in1=z[:].unsqueeze(2).broadcast_to([128, 4, 64]),
                    op=mybir.AluOpType.mult)
                attnT = []
                for pr in range(2):
                    tp_ps = tp_ps_pool.tile([128, 128], BF16, name="tp_ps", tag="tp")
                    nc.tensor.transpose(tp_ps[:], attn[:, pr * 128:(pr + 1) * 128],
                                        ident[:])
                    aT = wk2.tile([128, 128], BF16, name="aT", tag="aT", bufs=6)
                    nc.vector.tensor_copy(aT[:], tp_ps[:])
                    attnT.append(aT)

                out_sb = wk2.tile([128, E], BF16, name="out_sb", tag="out_sb", bufs=3)
                for nck in range(2):
                    op_ps = op_ps_pool.tile([128, 512], F32, name="op_ps", tag="op")
                    for pr in range(2):
                        nc.tensor.matmul(
                            op_ps[:],
                            attnT[pr][:],
                            wo3[:, pr, nck * 512:(nck + 1) * 512],
                            start=(pr == 0), stop=(pr == 1),
                        )
                    nc.scalar.activation(out_sb[:, nck * 512:(nck + 1) * 512], op_ps[:],
                                         mybir.ActivationFunctionType.Copy)
                nc.sync.dma_start(out_d[lsl, :], out_sb[:])

    nc.compile()
    return nc


def host_prep(query, cos, sin, Wq, Wk, Wv, Wo, L=L_FULL, LT=32):
    """Build per-core input maps (fp16 compute payloads, bf16 out-proj)."""
    F16N = np.float16
    cos2 = np.ascontiguousarray(cos[0], dtype=np.float32)   # (L, D)
    sin2 = np.ascontiguousarray(sin[0], dtype=np.float32)
    cosT_rep = np.ascontiguousarray(np.tile(cos2.T, (2, 1))).astype(F16N)  # (128, L)
    sinT_rep = np.ascontiguousarray(np.tile(sin2.T, (2, 1))).astype(F16N)
    cosL = np.ascontiguousarray(
        cos2.reshape(LT, 128, D).transpose(1, 0, 2).reshape(128, LT * D)).astype(F16N)
    sinm2 = np.concatenate([-sin2[:, :D // 2], sin2[:, D // 2:]], axis=1)
    sinmL = np.ascontiguousarray(
        sinm2.reshape(LT, 128, D).transpose(1, 0, 2).reshape(128, LT * D)).astype(F16N)

    idx = (np.pi / 2) * np.arange(1, L + 1, dtype=np.float32) / L
    s_arr = np.sin(idx).astype(np.float32)
    c_arr = np.cos(idx).astype(np.float32)
    sc = np.empty((128, LT * 2), dtype=np.float32)
    sc[:, 0::2] = s_arr.reshape(LT, 128).T
    sc[:, 1::2] = c_arr.reshape(LT, 128).T
    sT_rep = np.ascontiguousarray(np.tile(s_arr[None, :], (128, 1))).astype(F16N)
    cT_rep = np.ascontiguousarray(np.tile(c_arr[None, :], (128, 1))).astype(F16N)

    p_rot = np.zeros((D, D), dtype=np.float32)
    for j in range(D // 2):
        p_rot[D // 2 + j, j] = -1.0   # rot[:, j] = -q[:, 32+j]
        p_rot[j, D // 2 + j] = 1.0    # rot[:, 32+j] = q[:, j]
    p2 = np.zeros((128, 128), dtype=np.float32)
    p2[0:64, 0:64] = p_rot
    p2[64:128, 64:128] = p_rot

    qbT = [np.ascontiguousarray(query[:, b, :].T).astype(F16N) for b in range(N_BATCH)]

    in_maps = []
    for c in range(N_CORES):
        b = c // 4
        r0 = (c % 4) * F_LOC
        in_maps.append({
            "qbT": qbT[b],
            "wqT": np.ascontiguousarray(Wq[r0:r0 + F_LOC, :].T).astype(F16N),
            "wkvT": np.ascontiguousarray(
                np.concatenate([Wk[r0:r0 + F_LOC, :].T, Wv[r0:r0 + F_LOC, :].T],
                               axis=1)).astype(F16N),
            "wo_rhs": np.ascontiguousarray(Wo[:, r0:r0 + F_LOC].T).astype(BF),
            "cosT_rep": cosT_rep,
            "sinT_rep": sinT_rep,
            "sT_rep": sT_rep,
            "cT_rep": cT_rep,
            "cosL": cosL,
            "sinmL": sinmL,
            "sc_t": sc,
            "sc_tr": sc.astype(F16N),
            "p2_rot": p2.astype(F16N),
            "ident_r": np.eye(128, dtype=np.float32).astype(BF),
        })
    return in_maps


_PROG_CACHE = {}


def run(inputs, trace=False, trace_kwargs=None):
    """Run on 8 NeuronCores; returns (output, BassKernelResults)."""
    from concourse.bass_utils import run_bass_kernel_spmd

    LT = L_FULL // 128
    if LT not in _PROG_CACHE:
        _PROG_CACHE[LT] = build_program(LT)
    nc = _PROG_CACHE[LT]
    in_maps = host_prep(**inputs)
    kw = {}
    if trace:
        kw["trace"] = True
        if trace_kwargs:
            kw.update(trace_kwargs)
    res = run_bass_kernel_spmd(nc, in_maps, core_ids=list(range(N_CORES)), **kw)
    partials = [res.results[c]["out_partial"] for c in range(N_CORES)]
    out = np.empty((L_FULL, N_BATCH, E), dtype=np.float32)
    for b in range(N_BATCH):
        acc = partials[4 * b].astype(np.float32)
        for c in range(4 * b + 1, 4 * b + 4):
            acc += partials[c].astype(np.float32)
        out[:, b, :] = acc
    return out, res


def kernel(**inputs):
    out, _ = run(inputs)
    return out


# revision 9
# speedup vs baseline: 1.8881x; 1.8881x over previous
"""Cosformer attention Trainium2 kernel (fp16 compute / bf16 post-z edition).

Shards batch*heads across 8 NeuronCores: core c handles batch c//4 and
heads 4*(c%4) .. 4*(c%4)+4 (a 256-wide slice of the embedding). Each core:
  - projects q/k/v from its batch's query slice (fp16 matmuls, fp32 PSUM),
  - applies RoPE + relu; q' is pre-scaled by the cosformer sin/cos weights
    into two resident copies qs = q'*s_l and qc = q'*c_l (f-major),
  - computes the per-head cosformer kv summary (2D x D) + k-sums over all L,
  - pass 2: ab = qs@kv_top + qc@kv_bot accumulates numerator AND denominator
    in one PSUM group; z = 1/max(den,eps); attn scaled on gpsimd,
  - multiplies by its slice of Wo, producing a partial (L, E) output in bf16.
Host sums the 4 partials per batch in fp32. No cross-device communication.

Precision: the cosformer denominator crosses zero (min |den| ~ 2.8 vs median
~3800), so any 0.4%-level (bf16) noise upstream of z is catastrophic; fp16
(10-bit mantissa) matches the fp32r baseline's stability at half the DMA and
with the DVE 2x 16-bit mode. Post-z tensors (attn/out) overflow fp16's range
(z can be 1e6), so they are bf16. PSUM accumulation is fp32 throughout.
"""

import os
import sys

if "/opt/trn_rl_repo" not in sys.path:
    sys.path.insert(0, "/opt/trn_rl_repo")

from contextlib import ExitStack

import numpy as np
import ml_dtypes

import concourse.bass as bass
import concourse.bacc as bacc
import concourse.mybir as mybir
import concourse.tile as tile

F32 = mybir.dt.float32
BF16 = mybir.dt.bfloat16
F16 = mybir.dt.float16
BF = ml_dtypes.bfloat16
EPS = 1e-6

L_FULL, N_BATCH, E, H, D = 4096, 2, 1024, 16, 64
N_CORES = 8
HEADS_PER_CORE = 4          # 2 pairs
F_LOC = HEADS_PER_CORE * D  # 256


def build_program(LT=32):
    """Build the single-core SPMD Bass program. LT = number of 128-row L tiles."""
    L = LT * 128
    CH = 512                    # L columns per q-projection chunk
    NCH = L // CH
    LT_PER_CH = CH // 128

    nc = bacc.Bacc("TRN2", target_bir_lowering=False, debug=False)

    qbT_d = nc.dram_tensor("qbT", [E, L], F16, kind="ExternalInput").ap()
    wqT_d = nc.dram_tensor("wqT", [E, F_LOC], F16, kind="ExternalInput").ap()
    wkvT_d = nc.dram_tensor("wkvT", [E, 2 * F_LOC], F16, kind="ExternalInput").ap()
    wo_d = nc.dram_tensor("wo_rhs", [F_LOC, E], BF16, kind="ExternalInput").ap()
    cosT_d = nc.dram_tensor("cosT_rep", [128, L], F16, kind="ExternalInput").ap()
    sinT_d = nc.dram_tensor("sinT_rep", [128, L], F16, kind="ExternalInput").ap()
    sT_d = nc.dram_tensor("sT_rep", [128, L], F16, kind="ExternalInput").ap()
    cT_d = nc.dram_tensor("cT_rep", [128, L], F16, kind="ExternalInput").ap()
    cosL_d = nc.dram_tensor("cosL", [128, LT * D], F16, kind="ExternalInput").ap()
    sinmL_d = nc.dram_tensor("sinmL", [128, LT * D], F16, kind="ExternalInput").ap()
    sc_d = nc.dram_tensor("sc_t", [128, LT * 2], F32, kind="ExternalInput").ap()
    scr_d = nc.dram_tensor("sc_tr", [128, LT * 2], F16, kind="ExternalInput").ap()
    p2_d = nc.dram_tensor("p2_rot", [128, 128], F16, kind="ExternalInput").ap()
    id_d = nc.dram_tensor("ident_r", [128, 128], BF16, kind="ExternalInput").ap()
    out_d = nc.dram_tensor("out_partial", [L, E], BF16, kind="ExternalOutput").ap()

    with tile.TileContext(nc) as tc, ExitStack() as ctx:
        consts = ctx.enter_context(tc.tile_pool(name="consts", bufs=1))

        # ---- resident constants / inputs ----
        qc_sb = consts.tile([128, 8 * L], F16, name="qc_sb")       # whole input
        wq_sb = consts.tile([128, 8 * F_LOC], F16, name="wq_sb")
        wkv_sb = consts.tile([128, 8 * 2 * F_LOC], F16, name="wkv_sb")
        wo_sb = consts.tile([128, 2 * E], BF16, name="wo_sb")
        cosT_sb = consts.tile([128, L], F16, name="cosT_sb")
        sinT_sb = consts.tile([128, L], F16, name="sinT_sb")
        sT_sb = consts.tile([128, L], F16, name="sT_sb")
        cT_sb = consts.tile([128, L], F16, name="cT_sb")
        cosL_sb = consts.tile([128, LT * D], F16, name="cosL_sb")
        sinmL_sb = consts.tile([128, LT * D], F16, name="sinmL_sb")
        sc_sb = consts.tile([128, LT * 2], F32, name="sc_sb")
        scr_sb = consts.tile([128, LT * 2], F16, name="scr_sb")
        p2_sb = consts.tile([128, 128], F16, name="p2_sb")
        ident = consts.tile([128, 128], BF16, name="ident")

        qc3 = qc_sb[:].rearrange("p (e l) -> p e l", e=8)
        wq3 = wq_sb[:].rearrange("p (e f) -> p e f", e=8)
        wkv3 = wkv_sb[:].rearrange("p (e f) -> p e f", e=8)
        wo3 = wo_sb[:].rearrange("p (k j) -> p k j", k=2)
        qd3 = qbT_d.rearrange("(e p) l -> p e l", p=128)
        wqd3 = wqT_d.rearrange("(e p) f -> p e f", p=128)
        wkvd3 = wkvT_d.rearrange("(e p) f -> p e f", p=128)

        # resident cosformer-scaled q^T (f-major, per pair) and kv summaries
        qsT_sb = consts.tile([128, 2 * L], F16, name="qsT_sb")
        qcT_sb = consts.tile([128, 2 * L], F16, name="qcT_sb")
        qs3 = qsT_sb[:].rearrange("p (r l) -> p r l", r=2)
        qx3 = qcT_sb[:].rearrange("p (r l) -> p r l", r=2)
        kvtop_sb = [consts.tile([128, 130], F16, name=f"kvtop{pr}") for pr in range(2)]
        kvbot_sb = [consts.tile([128, 130], F16, name=f"kvbot{pr}") for pr in range(2)]

        # ---- DMA issue order: critical inputs first, spread across queues ----
        QQ = 4                       # input quarters (1024 L-cols each)
        LQ = L // QQ
        for e in range(8):
            nc.sync.dma_start(qc3[:, e:e + 1, 0:LQ], qd3[:, e:e + 1, 0:LQ])
            nc.scalar.dma_start(wq3[:, e:e + 1, :], wqd3[:, e:e + 1, :])
            nc.gpsimd.dma_start(wkv3[:, e:e + 1, :], wkvd3[:, e:e + 1, :])
        nc.gpsimd.dma_start(cosL_sb[:], cosL_d[:])
        nc.gpsimd.dma_start(sinmL_sb[:], sinmL_d[:])
        nc.gpsimd.dma_start(sc_sb[:], sc_d[:])
        nc.gpsimd.dma_start(scr_sb[:], scr_d[:])
        nc.scalar.dma_start(p2_sb[:], p2_d[:])
        nc.scalar.dma_start(cosT_sb[:], cosT_d[:])
        nc.scalar.dma_start(sinT_sb[:], sinT_d[:])
        nc.scalar.dma_start(sT_sb[:], sT_d[:])
        nc.scalar.dma_start(cT_sb[:], cT_d[:])
        nc.gpsimd.dma_start(ident[:], id_d[:])
        for pr in range(2):
            nc.gpsimd.memset(kvtop_sb[pr][:], 0.0)
            nc.gpsimd.memset(kvbot_sb[pr][:], 0.0)
        for qq in range(1, QQ):
            for e in range(8):
                nc.sync.dma_start(qc3[:, e:e + 1, qq * LQ:(qq + 1) * LQ],
                                  qd3[:, e:e + 1, qq * LQ:(qq + 1) * LQ])
        nc.sync.dma_start(
            wo_sb[:].rearrange("p (k j) -> p k j", k=2),
            wo_d.rearrange("(k p) j -> p k j", p=128),
        )

        # ================= PASS 1 =================
        with ExitStack() as p1:
            proj_ps = p1.enter_context(tc.tile_pool(name="proj_ps", bufs=3, space="PSUM"))
            q_ps_pool = p1.enter_context(tc.tile_pool(name="q_ps", bufs=3, space="PSUM"))
            kv_ps_pool = p1.enter_context(tc.tile_pool(name="kv_ps", bufs=1, space="PSUM"))
            wk1 = p1.enter_context(tc.tile_pool(name="wk1", bufs=3))

            kv_ps = [kv_ps_pool.tile([128, 258], F32, name=f"kv_ps{pr}") for pr in range(2)]

            for ch in range(NCH):
                csl = slice(ch * CH, (ch + 1) * CH)

                # ---- q: f-major projection + relu + RoPE + s/c scale (per pair) ----
                for pr in range(2):
                    qT_ps = q_ps_pool.tile([128, CH], F32, name="qT_ps", tag="q512")
                    for e in range(8):
                        nc.tensor.matmul(
                            qT_ps[:],
                            wq3[:, e, pr * 128:(pr + 1) * 128],
                            qc3[:, e, csl],
                            start=(e == 0), stop=(e == 7),
                        )
                    qTr = wk1.tile([128, CH], F16, name="qTr", tag="qTr")
                    nc.scalar.activation(qTr[:], qT_ps[:], mybir.ActivationFunctionType.Relu)
                    rot_ps = q_ps_pool.tile([128, CH], F32, name="rot_ps", tag="q512")
                    nc.tensor.matmul(
                        rot_ps[:], p2_sb[:], qTr[:],
                        start=True, stop=True,
                    )
                    tq = wk1.tile([128, CH], F16, name="tq", tag="tq")
                    nc.vector.tensor_tensor(
                        out=tq[:], in0=qTr[:], in1=cosT_sb[:, csl], op=mybir.AluOpType.mult)
                    uq = wk1.tile([128, CH], F16, name="uq", tag="uq")
                    nc.vector.tensor_tensor(
                        out=uq[:], in0=rot_ps[:], in1=sinT_sb[:, csl], op=mybir.AluOpType.mult)
                    qq_t = wk1.tile([128, CH], F16, name="qq_t", tag="qq_t")
                    nc.vector.tensor_tensor(
                        out=qq_t[:], in0=tq[:], in1=uq[:], op=mybir.AluOpType.add)
                    nc.vector.tensor_tensor(
                        out=qs3[:, pr, csl], in0=qq_t[:], in1=sT_sb[:, csl],
                        op=mybir.AluOpType.mult)
                    nc.vector.tensor_tensor(
                        out=qx3[:, pr, csl], in0=qq_t[:], in1=cT_sb[:, csl],
                        op=mybir.AluOpType.mult)

                # ---- k, v: L-major projections, per L-tile ----
                for lt in range(LT_PER_CH):
                    t = ch * LT_PER_CH + lt
                    lsl = slice(t * 128, (t + 1) * 128)
                    kv_proj_ps = proj_ps.tile([128, 2 * F_LOC], F32, name="kv_proj_ps", tag="proj")
                    k_ps = kv_proj_ps[:, 0:F_LOC]
                    for e in range(8):
                        nc.tensor.matmul(kv_proj_ps[:], qc3[:, e, lsl], wkv3[:, e, :],
                                         start=(e == 0), stop=(e == 7))

                    # k: relu then RoPE (L-major; swap via reversed AP, signed sin)
                    k_sb = wk1.tile([128, F_LOC], F16, name="k_sb", tag="k_sb")
                    nc.scalar.activation(k_sb[:], k_ps, mybir.ActivationFunctionType.Relu)
                    dsl = slice(t * D, (t + 1) * D)
                    cosL_t = cosL_sb[:, dsl].rearrange("p (a j) -> p a j", a=2) \
                        .unsqueeze(1).broadcast_to([128, 4, 2, 32])
                    sinm_t = sinmL_sb[:, dsl].rearrange("p (a j) -> p a j", a=2) \
                        .unsqueeze(1).broadcast_to([128, 4, 2, 32])
                    kt1 = wk1.tile([128, F_LOC], F16, name="kt1", tag="kt1")
                    nc.vector.tensor_tensor(
                        out=kt1[:].rearrange("p (h a j) -> p h a j", h=4, a=2),
                        in0=k_sb[:].rearrange("p (h a j) -> p h a j", h=4, a=2),
                        in1=cosL_t, op=mybir.AluOpType.mult)
                    kt2 = wk1.tile([128, F_LOC], F16, name="kt2", tag="kt2")
                    nc.vector.tensor_tensor(
                        out=kt2[:].rearrange("p (h a j) -> p h a j", h=4, a=2),
                        in0=k_sb[:].rearrange("p (h a j) -> p h a j", h=4, a=2)[:, :, ::-1, :],
                        in1=sinm_t, op=mybir.AluOpType.mult)
                    nc.vector.tensor_tensor(out=kt1[:], in0=kt1[:], in1=kt2[:],
                                            op=mybir.AluOpType.add)

                    # v: vs scaled on Act, vc scaled on DVE (both read PSUM);
                    # tiny s/c cols appended by gpsimd (SBUF->SBUF copy only)
                    vsc = wk1.tile([128, 520], F16, name="vsc", tag="vsc")
                    s_col = sc_sb[:, 2 * t:2 * t + 1]
                    c_col = sc_sb[:, 2 * t + 1:2 * t + 2]
                    for pr in range(2):
                        base = pr * 260
                        vp = kv_proj_ps[:, F_LOC + pr * 128:F_LOC + (pr + 1) * 128]
                        nc.scalar.activation(vsc[:, base:base + 128], vp,
                                             mybir.ActivationFunctionType.Copy, scale=s_col)
                        nc.vector.tensor_scalar_mul(vsc[:, base + 128:base + 256], vp, c_col)
                        nc.gpsimd.tensor_copy(vsc[:, base + 256:base + 258],
                                              scr_sb[:, 2 * t:2 * t + 2])

                    # kv accumulation (single group per pair incl k-sums)
                    for pr in range(2):
                        psl = slice(pr * 128, (pr + 1) * 128)
                        nc.tensor.matmul(
                            kv_ps[pr][:],
                            kt1[:, psl],
                            vsc[:, pr * 260:pr * 260 + 258],
                            start=(t == 0), stop=(t == LT - 1),
                        )

            # ---- kv eviction / per-head rearrangement ----
            # kv_ps[pr]: rows 0:64 = head A (d), 64:128 = head B;
            # cols 0:128 kv_top (A cols 0:64, B cols 64:128), 128:256 kv_bot, 256:258 ksum t/b.
            # kvtop_sb[pr] block-diagonal (128, 130): rows 0:64 (head A d) hold A's
            # [kvtop|kstop] in cols 0:65; rows 64:128 hold B's in cols 65:130.
            CP = mybir.ActivationFunctionType.Copy
            for pr in range(2):
                nc.vector.tensor_copy(kvtop_sb[pr][0:64, 0:64], kv_ps[pr][0:64, 0:64])
                nc.vector.tensor_copy(kvtop_sb[pr][0:64, 64:65], kv_ps[pr][0:64, 256:257])
                nc.vector.tensor_copy(kvtop_sb[pr][64:128, 65:129], kv_ps[pr][64:128, 64:128])
                nc.vector.tensor_copy(kvtop_sb[pr][64:128, 129:130], kv_ps[pr][64:128, 256:257])
                nc.scalar.activation(kvbot_sb[pr][0:64, 0:64], kv_ps[pr][0:64, 128:192], CP)
                nc.scalar.activation(kvbot_sb[pr][0:64, 64:65], kv_ps[pr][0:64, 257:258], CP)
                nc.scalar.activation(kvbot_sb[pr][64:128, 65:129], kv_ps[pr][64:128, 192:256], CP)
                nc.scalar.activation(kvbot_sb[pr][64:128, 129:130], kv_ps[pr][64:128, 257:258], CP)

        # ================= PASS 2 =================
        with ExitStack() as p2:
            ab_ps_pool = p2.enter_context(tc.tile_pool(name="ab_ps", bufs=3, space="PSUM"))
            tp_ps_pool = p2.enter_context(tc.tile_pool(name="tp_ps", bufs=2, space="PSUM"))
            op_ps_pool = p2.enter_context(tc.tile_pool(name="op_ps", bufs=3, space="PSUM"))
            wk2 = p2.enter_context(tc.tile_pool(name="wk2", bufs=3))

            for t in range(LT):
                l0 = t * 128
                lsl = slice(l0, l0 + 128)
                # ab holds both pairs: cols 0:130 = pr0 [Anum|Aden|Bnum|Bden], 130:260 = pr1.
                # Groups are strictly sequential per region (start..stop closed
                # before the next opens) -- interleaved open groups corrupt PSUM.
                ab_ps = ab_ps_pool.tile([128, 260], F32, name="ab_ps", tag="ab")
                for pr in range(2):
                    asl = slice(pr * 130, (pr + 1) * 130)
                    nc.tensor.matmul(
                        ab_ps[:, asl],
                        qs3[:, pr:pr + 1, lsl].squeeze(1),
                        kvtop_sb[pr][:],
                        start=True, stop=False,
                    )
                    nc.tensor.matmul(
                        ab_ps[:, asl],
                        qx3[:, pr:pr + 1, lsl].squeeze(1),
                        kvbot_sb[pr][:],
                        start=False, stop=True,
                    )
                ab4 = ab_ps[:].rearrange("p (h x) -> p h x", h=4)
                z = wk2.tile([128, 4], F32, name="z", tag="z")
                nc.vector.tensor_scalar_max(z[:], ab4[:, :, 64], EPS)
                nc.vector.reciprocal(z[:], z[:])
                attn = wk2.tile([128, 256], BF16, name="attn", tag="attn")
                nc.vector.tensor_tensor(
                    out=attn[:].rearrange("p (h j) -> p h j", h=4),
                    in0=ab4[:, :, 0:64],
                    in1=z[:].unsqueeze(2).broadcast_to([128, 4, 64]),
                    op=mybir.AluOpType.mult)
                attnT = []
                for pr in range(2):
                    tp_ps = tp_ps_pool.tile([128, 128], BF16, name="tp_ps", tag="tp")
                    nc.tensor.transpose(tp_ps[:], attn[:, pr * 128:(pr + 1) * 128],
                                        ident[:])
                    aT = wk2.tile([128, 128], BF16, name="aT", tag="aT", bufs=6)
                    nc.vector.tensor_copy(aT[:], tp_ps[:])
                    attnT.append(aT)

                out_sb = wk2.tile([128, E], BF16, name="out_sb", tag="out_sb", bufs=3)
                for nck in range(2):
                    op_ps = op_ps_pool.tile([128, 512], F32, name="op_ps", tag="op")
                    for pr in range(2):
                        nc.tensor.matmul(
                            op_ps[:],
                            attnT[pr][:],
                            wo3[:, pr, nck * 512:(nck + 1) * 512],
                            start=(pr == 0), stop=(pr == 1),
                        )
                    nc.scalar.activation(out_sb[:, nck * 512:(nck + 1) * 512], op_ps[:],
                                         mybir.ActivationFunctionType.Copy)
                nc.sync.dma_start(out_d[lsl, :], out_sb[:])

    nc.compile()
    return nc


def host_prep(query, cos, sin, Wq, Wk, Wv, Wo, L=L_FULL, LT=32):
    """Build per-core input maps (fp16 compute payloads, bf16 out-proj)."""
    F16N = np.float16
    cos2 = np.ascontiguousarray(cos[0], dtype=np.float32)   # (L, D)
    sin2 = np.ascontiguousarray(sin[0], dtype=np.float32)
    cosT_rep = np.ascontiguousarray(np.tile(cos2.T, (2, 1))).astype(F16N)  # (128, L)
    sinT_rep = np.ascontiguousarray(np.tile(sin2.T, (2, 1))).astype(F16N)
    cosL = np.ascontiguousarray(
        cos2.reshape(LT, 128, D).transpose(1, 0, 2).reshape(128, LT * D)).astype(F16N)
    sinm2 = np.concatenate([-sin2[:, :D // 2], sin2[:, D // 2:]], axis=1)
    sinmL = np.ascontiguousarray(
        sinm2.reshape(LT, 128, D).transpose(1, 0, 2).reshape(128, LT * D)).astype(F16N)

    idx = (np.pi / 2) * np.arange(1, L + 1, dtype=np.float32) / L
    s_arr = np.sin(idx).astype(np.float32)
    c_arr = np.cos(idx).astype(np.float32)
    sc = np.empty((128, LT * 2), dtype=np.float32)
    sc[:, 0::2] = s_arr.reshape(LT, 128).T
    sc[:, 1::2] = c_arr.reshape(LT, 128).T
    sT_rep = np.ascontiguousarray(np.tile(s_arr[None, :], (128, 1))).astype(F16N)
    cT_rep = np.ascontiguousarray(np.tile(c_arr[None, :], (128, 1))).astype(F16N)

    p_rot = np.zeros((D, D), dtype=np.float32)
    for j in range(D // 2):
        p_rot[D // 2 + j, j] = -1.0   # rot[:, j] = -q[:, 32+j]
        p_rot[j, D // 2 + j] = 1.0    # rot[:, 32+j] = q[:, j]
    p2 = np.zeros((128, 128), dtype=np.float32)
    p2[0:64, 0:64] = p_rot
    p2[64:128, 64:128] = p_rot

    qbT = [np.ascontiguousarray(query[:, b, :].T).astype(F16N) for b in range(N_BATCH)]

    in_maps = []
    for c in range(N_CORES):
        b = c // 4
        r0 = (c % 4) * F_LOC
        in_maps.append({
            "qbT": qbT[b],
            "wqT": np.ascontiguousarray(Wq[r0:r0 + F_LOC, :].T).astype(F16N),
            "wkvT": np.ascontiguousarray(
                np.concatenate([Wk[r0:r0 + F_LOC, :].T, Wv[r0:r0 + F_LOC, :].T],
                               axis=1)).astype(F16N),
            "wo_rhs": np.ascontiguousarray(Wo[:, r0:r0 + F_LOC].T).astype(BF),
            "cosT_rep": cosT_rep,
            "sinT_rep": sinT_rep,
            "sT_rep": sT_rep,
            "cT_rep": cT_rep,
            "cosL": cosL,
            "sinmL": sinmL,
            "sc_t": sc,
            "sc_tr": sc.astype(F16N),
            "p2_rot": p2.astype(F16N),
            "ident_r": np.eye(128, dtype=np.float32).astype(BF),
        })
    return in_maps


_PROG_CACHE = {}


def run(inputs, trace=False, trace_kwargs=None):
    """Run on 8 NeuronCores; returns (output, BassKernelResults)."""
    from concourse.bass_utils import run_bass_kernel_spmd

    LT = L_FULL // 128
    if LT not in _PROG_CACHE:
        _PROG_CACHE[LT] = build_program(LT)
    nc = _PROG_CACHE[LT]
    in_maps = host_prep(**inputs)
    kw = {}
    if trace:
        kw["trace"] = True
        if trace_kwargs:
            kw.update(trace_kwargs)
    res = run_bass_kernel_spmd(nc, in_maps, core_ids=list(range(N_CORES)), **kw)
    partials = [res.results[c]["out_partial"] for c in range(N_CORES)]
    out = np.empty((L_FULL, N_BATCH, E), dtype=np.float32)
    for b in range(N_BATCH):
        acc = partials[4 * b].astype(np.float32)
        for c in range(4 * b + 1, 4 * b + 4):
            acc += partials[c].astype(np.float32)
        out[:, b, :] = acc
    return out, res


def kernel(**inputs):
    out, _ = run(inputs)
    return out


# revision 11
# speedup vs baseline: 2.0614x; 1.0918x over previous
"""Cosformer attention Trainium2 kernel (fp16 compute / bf16 post-z edition).

Shards batch*heads across 8 NeuronCores: core c handles batch c//4 and
heads 4*(c%4) .. 4*(c%4)+4 (a 256-wide slice of the embedding). Each core:
  - projects q/k/v from its batch's query slice (fp16 matmuls, fp32 PSUM),
  - applies RoPE + relu; q' is pre-scaled by the cosformer sin/cos weights
    into two resident copies qs = q'*s_l and qc = q'*c_l (f-major),
  - computes the per-head cosformer kv summary (2D x D) + k-sums over all L,
  - pass 2: ab = qs@kv_top + qc@kv_bot accumulates numerator AND denominator
    in one PSUM group; z = 1/max(den,eps); attn scaled on gpsimd,
  - multiplies by its slice of Wo, producing a partial (L, E) output in bf16.
Host sums the 4 partials per batch in fp32. No cross-device communication.

Precision: the cosformer denominator crosses zero (min |den| ~ 2.8 vs median
~3800), so any 0.4%-level (bf16) noise upstream of z is catastrophic; fp16
(10-bit mantissa) matches the fp32r baseline's stability at half the DMA and
with the DVE 2x 16-bit mode. Post-z tensors (attn/out) overflow fp16's range
(z can be 1e6), so they are bf16. PSUM accumulation is fp32 throughout.
"""

import os
import sys

if "/opt/trn_rl_repo" not in sys.path:
    sys.path.insert(0, "/opt/trn_rl_repo")

from contextlib import ExitStack

import numpy as np
import ml_dtypes

import concourse.bass as bass
import concourse.bacc as bacc
import concourse.mybir as mybir
import concourse.tile as tile

F32 = mybir.dt.float32
BF16 = mybir.dt.bfloat16
F16 = mybir.dt.float16
BF = ml_dtypes.bfloat16
EPS = 1e-6

L_FULL, N_BATCH, E, H, D = 4096, 2, 1024, 16, 64
N_CORES = 8
HEADS_PER_CORE = 4          # 2 pairs
F_LOC = HEADS_PER_CORE * D  # 256


def build_program(LT=32):
    """Build the single-core SPMD Bass program. LT = number of 128-row L tiles."""
    L = LT * 128
    CH = 512                    # L columns per q-projection chunk
    NCH = L // CH
    LT_PER_CH = CH // 128

    nc = bacc.Bacc("TRN2", target_bir_lowering=False, debug=False)

    qbT_d = nc.dram_tensor("qbT", [E, L], F16, kind="ExternalInput").ap()
    wqT_d = nc.dram_tensor("wqT", [E, F_LOC], F16, kind="ExternalInput").ap()
    wkvT_d = nc.dram_tensor("wkvT", [E, 2 * F_LOC], F16, kind="ExternalInput").ap()
    wo_d = nc.dram_tensor("wo_rhs", [F_LOC, E], BF16, kind="ExternalInput").ap()
    cosT_d = nc.dram_tensor("cosT_rep", [128, L], F16, kind="ExternalInput").ap()
    sinT_d = nc.dram_tensor("sinT_rep", [128, L], F16, kind="ExternalInput").ap()
    sT_d = nc.dram_tensor("sT_rep", [128, L], F16, kind="ExternalInput").ap()
    cT_d = nc.dram_tensor("cT_rep", [128, L], F16, kind="ExternalInput").ap()
    cosL_d = nc.dram_tensor("cosL", [128, LT * D], F16, kind="ExternalInput").ap()
    sinmL_d = nc.dram_tensor("sinmL", [128, LT * D], F16, kind="ExternalInput").ap()
    sc_d = nc.dram_tensor("sc_t", [128, LT * 2], F32, kind="ExternalInput").ap()
    scr_d = nc.dram_tensor("sc_tr", [128, LT * 2], F16, kind="ExternalInput").ap()
    p2_d = nc.dram_tensor("p2_rot", [128, 128], F16, kind="ExternalInput").ap()
    id_d = nc.dram_tensor("ident_r", [128, 128], BF16, kind="ExternalInput").ap()
    sel_d = nc.dram_tensor("sel_r", [4, 256], mybir.dt.float32r, kind="ExternalInput").ap()
    out_d = nc.dram_tensor("out_partial", [L, E], BF16, kind="ExternalOutput").ap()

    with tile.TileContext(nc) as tc, ExitStack() as ctx:
        consts = ctx.enter_context(tc.tile_pool(name="consts", bufs=1))

        # ---- resident constants / inputs ----
        qc_sb = consts.tile([128, 8 * L], F16, name="qc_sb")       # whole input
        wq_sb = consts.tile([128, 8 * F_LOC], F16, name="wq_sb")
        wkv_sb = consts.tile([128, 8 * 2 * F_LOC], F16, name="wkv_sb")
        wo_sb = consts.tile([128, 2 * E], BF16, name="wo_sb")
        cosT_sb = consts.tile([128, L], F16, name="cosT_sb")
        sinT_sb = consts.tile([128, L], F16, name="sinT_sb")
        sT_sb = consts.tile([128, L], F16, name="sT_sb")
        cT_sb = consts.tile([128, L], F16, name="cT_sb")
        cosL_sb = consts.tile([128, LT * D], F16, name="cosL_sb")
        sinmL_sb = consts.tile([128, LT * D], F16, name="sinmL_sb")
        sc_sb = consts.tile([128, LT * 2], F32, name="sc_sb")
        scr_sb = consts.tile([128, LT * 2], F16, name="scr_sb")
        p2_sb = consts.tile([128, 128], F16, name="p2_sb")
        ident = consts.tile([128, 128], BF16, name="ident")

        qc3 = qc_sb[:].rearrange("p (e l) -> p e l", e=8)
        wq3 = wq_sb[:].rearrange("p (e f) -> p e f", e=8)
        wkv3 = wkv_sb[:].rearrange("p (e f) -> p e f", e=8)
        wo3 = wo_sb[:].rearrange("p (k j) -> p k j", k=2)
        qd3 = qbT_d.rearrange("(e p) l -> p e l", p=128)
        wqd3 = wqT_d.rearrange("(e p) f -> p e f", p=128)
        wkvd3 = wkvT_d.rearrange("(e p) f -> p e f", p=128)

        # resident cosformer-scaled q^T (f-major, per pair) and kv summaries
        qsT_sb = consts.tile([128, 2 * L], F16, name="qsT_sb")
        qcT_sb = consts.tile([128, 2 * L], F16, name="qcT_sb")
        qs3 = qsT_sb[:].rearrange("p (r l) -> p r l", r=2)
        qx3 = qcT_sb[:].rearrange("p (r l) -> p r l", r=2)
        kvtop_sb = [consts.tile([128, 128], F16, name=f"kvtop{pr}") for pr in range(2)]
        kvbot_sb = [consts.tile([128, 128], F16, name=f"kvbot{pr}") for pr in range(2)]
        ks_sb = [consts.tile([128, 4], F16, name=f"ks_sb{pr}") for pr in range(2)]
        sel_sb = consts.tile([4, 256], mybir.dt.float32r, name="sel_sb")

        # ---- DMA issue order: critical inputs first, spread across queues ----
        # sync: input quarters + small L-tile consts; scalar: weights + q-RoPE trig;
        # gp: remaining consts + memsets. First kv tile needs cosL/sc/scr early.
        QQ = 4                       # input quarters (1024 L-cols each)
        LQ = L // QQ
        for e in range(8):
            nc.sync.dma_start(qc3[:, e:e + 1, 0:LQ], qd3[:, e:e + 1, 0:LQ])
            nc.scalar.dma_start(wq3[:, e:e + 1, :], wqd3[:, e:e + 1, :])
            nc.gpsimd.dma_start(wkv3[:, e:e + 1, :], wkvd3[:, e:e + 1, :])
        nc.sync.dma_start(cosL_sb[:], cosL_d[:])
        nc.sync.dma_start(sinmL_sb[:], sinmL_d[:])
        nc.sync.dma_start(sc_sb[:], sc_d[:])
        nc.sync.dma_start(scr_sb[:], scr_d[:])
        nc.scalar.dma_start(p2_sb[:], p2_d[:])
        nc.scalar.dma_start(cosT_sb[:], cosT_d[:])
        nc.scalar.dma_start(sinT_sb[:], sinT_d[:])
        nc.gpsimd.dma_start(sT_sb[:], sT_d[:])
        nc.gpsimd.dma_start(cT_sb[:], cT_d[:])
        nc.gpsimd.dma_start(ident[:], id_d[:])
        nc.gpsimd.dma_start(sel_sb[:], sel_d[:])
        for pr in range(2):
            nc.gpsimd.memset(kvtop_sb[pr][:], 0.0)
            nc.gpsimd.memset(kvbot_sb[pr][:], 0.0)
            nc.gpsimd.memset(ks_sb[pr][:], 0.0)
        for qq in range(1, QQ):
            for e in range(8):
                nc.sync.dma_start(qc3[:, e:e + 1, qq * LQ:(qq + 1) * LQ],
                                  qd3[:, e:e + 1, qq * LQ:(qq + 1) * LQ])
        nc.sync.dma_start(
            wo_sb[:].rearrange("p (k j) -> p k j", k=2),
            wo_d.rearrange("(k p) j -> p k j", p=128),
        )

        # ================= PASS 1 =================
        with ExitStack() as p1:
            proj_ps = p1.enter_context(tc.tile_pool(name="proj_ps", bufs=3, space="PSUM"))
            q_ps_pool = p1.enter_context(tc.tile_pool(name="q_ps", bufs=3, space="PSUM"))
            kv_ps_pool = p1.enter_context(tc.tile_pool(name="kv_ps", bufs=1, space="PSUM"))
            wk1 = p1.enter_context(tc.tile_pool(name="wk1", bufs=3))

            kv_ps = [kv_ps_pool.tile([128, 258], F32, name=f"kv_ps{pr}") for pr in range(2)]

            for ch in range(NCH):
                csl = slice(ch * CH, (ch + 1) * CH)

                # ---- q: f-major projection + relu + RoPE + s/c scale (per pair) ----
                for pr in range(2 if True else 0):
                    qT_ps = q_ps_pool.tile([128, CH], F32, name="qT_ps", tag="q512")
                    for e in range(8):
                        nc.tensor.matmul(
                            qT_ps[:],
                            wq3[:, e, pr * 128:(pr + 1) * 128],
                            qc3[:, e, csl],
                            start=(e == 0), stop=(e == 7),
                        )
                    qTr = wk1.tile([128, CH], F16, name="qTr", tag="qTr")
                    nc.scalar.activation(qTr[:], qT_ps[:], mybir.ActivationFunctionType.Relu)
                    rot_ps = q_ps_pool.tile([128, CH], F32, name="rot_ps", tag="q512")
                    nc.tensor.matmul(
                        rot_ps[:], p2_sb[:], qTr[:],
                        start=True, stop=True,
                    )
                    tq = wk1.tile([128, CH], F16, name="tq", tag="tq")
                    nc.vector.tensor_tensor(
                        out=tq[:], in0=qTr[:], in1=cosT_sb[:, csl], op=mybir.AluOpType.mult)
                    uq = wk1.tile([128, CH], F16, name="uq", tag="uq")
                    nc.vector.tensor_tensor(
                        out=uq[:], in0=rot_ps[:], in1=sinT_sb[:, csl], op=mybir.AluOpType.mult)
                    qq_t = wk1.tile([128, CH], F16, name="qq_t", tag="qq_t")
                    nc.vector.tensor_tensor(
                        out=qq_t[:], in0=tq[:], in1=uq[:], op=mybir.AluOpType.add)
                    nc.vector.tensor_tensor(
                        out=qs3[:, pr, csl], in0=qq_t[:], in1=sT_sb[:, csl],
                        op=mybir.AluOpType.mult)
                    nc.vector.tensor_tensor(
                        out=qx3[:, pr, csl], in0=qq_t[:], in1=cT_sb[:, csl],
                        op=mybir.AluOpType.mult)

                # ---- k, v: L-major projections, per L-tile ----
                for lt in range(LT_PER_CH):
                    t = ch * LT_PER_CH + lt
                    lsl = slice(t * 128, (t + 1) * 128)
                    kv_proj_ps = proj_ps.tile([128, 2 * F_LOC], F32, name="kv_proj_ps", tag="proj")
                    k_ps = kv_proj_ps[:, 0:F_LOC]
                    for e in range(8):
                        nc.tensor.matmul(kv_proj_ps[:], qc3[:, e, lsl], wkv3[:, e, :],
                                         start=(e == 0), stop=(e == 7))

                    # k: relu then RoPE (L-major; swap via reversed AP, signed sin)
                    k_sb = wk1.tile([128, F_LOC], F16, name="k_sb", tag="k_sb")
                    nc.scalar.activation(k_sb[:], k_ps, mybir.ActivationFunctionType.Relu)
                    dsl = slice(t * D, (t + 1) * D)
                    cosL_t = cosL_sb[:, dsl].rearrange("p (a j) -> p a j", a=2) \
                        .unsqueeze(1).broadcast_to([128, 4, 2, 32])
                    sinm_t = sinmL_sb[:, dsl].rearrange("p (a j) -> p a j", a=2) \
                        .unsqueeze(1).broadcast_to([128, 4, 2, 32])
                    kt1 = wk1.tile([128, F_LOC], F16, name="kt1", tag="kt1")
                    nc.vector.tensor_tensor(
                        out=kt1[:].rearrange("p (h a j) -> p h a j", h=4, a=2),
                        in0=k_sb[:].rearrange("p (h a j) -> p h a j", h=4, a=2),
                        in1=cosL_t, op=mybir.AluOpType.mult)
                    kt2 = wk1.tile([128, F_LOC], F16, name="kt2", tag="kt2")
                    nc.vector.tensor_tensor(
                        out=kt2[:].rearrange("p (h a j) -> p h a j", h=4, a=2),
                        in0=k_sb[:].rearrange("p (h a j) -> p h a j", h=4, a=2)[:, :, ::-1, :],
                        in1=sinm_t, op=mybir.AluOpType.mult)
                    nc.vector.tensor_tensor(out=kt1[:], in0=kt1[:], in1=kt2[:],
                                            op=mybir.AluOpType.add)

                    # v: vs scaled on Act, vc scaled on DVE (both read PSUM);
                    # tiny s/c cols appended by gpsimd (SBUF->SBUF copy only)
                    vsc = wk1.tile([128, 520], F16, name="vsc", tag="vsc")
                    s_col = sc_sb[:, 2 * t:2 * t + 1]
                    c_col = sc_sb[:, 2 * t + 1:2 * t + 2]
                    for pr in range(2):
                        base = pr * 260
                        vp = kv_proj_ps[:, F_LOC + pr * 128:F_LOC + (pr + 1) * 128]
                        nc.scalar.activation(vsc[:, base:base + 128], vp,
                                             mybir.ActivationFunctionType.Copy, scale=s_col)
                        nc.vector.tensor_scalar_mul(vsc[:, base + 128:base + 256], vp, c_col)
                        nc.gpsimd.tensor_copy(vsc[:, base + 256:base + 258],
                                              scr_sb[:, 2 * t:2 * t + 2])

                    # kv accumulation (single group per pair incl k-sums)
                    for pr in range(2):
                        psl = slice(pr * 128, (pr + 1) * 128)
                        nc.tensor.matmul(
                            kv_ps[pr][:],
                            kt1[:, psl],
                            vsc[:, pr * 260:pr * 260 + 258],
                            start=(t == 0), stop=(t == LT - 1),
                        )

            # ---- kv eviction / per-head rearrangement ----
            # kv_ps[pr]: rows 0:64 = head A (d), 64:128 = head B;
            # cols 0:128 kv_top (A cols 0:64, B cols 64:128), 128:256 kv_bot, 256:258 ksum t/b.
            # kvtop_sb[pr] block-diagonal (128, 130): rows 0:64 (head A d) hold A's
            # [kvtop|kstop] in cols 0:65; rows 64:128 hold B's in cols 65:130.
            CP = mybir.ActivationFunctionType.Copy
            for pr in range(2):
                nc.vector.tensor_copy(kvtop_sb[pr][0:64, 0:64], kv_ps[pr][0:64, 0:64])
                nc.vector.tensor_copy(kvtop_sb[pr][0:64, 64:65], kv_ps[pr][0:64, 256:257])
                nc.vector.tensor_copy(kvtop_sb[pr][64:128, 65:129], kv_ps[pr][64:128, 64:128])
                nc.vector.tensor_copy(kvtop_sb[pr][64:128, 129:130], kv_ps[pr][64:128, 256:257])
                nc.scalar.activation(kvbot_sb[pr][0:64, 0:64], kv_ps[pr][0:64, 128:192], CP)
                nc.scalar.activation(kvbot_sb[pr][0:64, 64:65], kv_ps[pr][0:64, 257:258], CP)
                nc.scalar.activation(kvbot_sb[pr][64:128, 65:129], kv_ps[pr][64:128, 192:256], CP)
                nc.scalar.activation(kvbot_sb[pr][64:128, 129:130], kv_ps[pr][64:128, 257:258], CP)

        # ================= PASS 2 =================
        with ExitStack() as p2:
            ab_ps_pool = p2.enter_context(tc.tile_pool(name="ab_ps", bufs=3, space="PSUM"))
            tp_ps_pool = p2.enter_context(tc.tile_pool(name="tp_ps", bufs=2, space="PSUM"))
            op_ps_pool = p2.enter_context(tc.tile_pool(name="op_ps", bufs=3, space="PSUM"))
            wk2 = p2.enter_context(tc.tile_pool(name="wk2", bufs=3))

            for t in range(LT):
                l0 = t * 128
                lsl = slice(l0, l0 + 128)
                # ab holds both pairs: cols 0:130 = pr0 [Anum|Aden|Bnum|Bden], 130:260 = pr1.
                # Groups are strictly sequential per region (start..stop closed
                # before the next opens) -- interleaved open groups corrupt PSUM.
                ab_ps = ab_ps_pool.tile([128, 260], F32, name="ab_ps", tag="ab")
                for pr in range(2):
                    asl = slice(pr * 130, (pr + 1) * 130)
                    nc.tensor.matmul(
                        ab_ps[:, asl],
                        qs3[:, pr:pr + 1, lsl].squeeze(1),
                        kvtop_sb[pr][:],
                        start=True, stop=False,
                    )
                    nc.tensor.matmul(
                        ab_ps[:, asl],
                        qx3[:, pr:pr + 1, lsl].squeeze(1),
                        kvbot_sb[pr][:],
                        start=False, stop=True,
                    )
                ab4 = ab_ps[:].rearrange("p (h x) -> p h x", h=4)
                z = wk2.tile([128, 4], F32, name="z", tag="z")
                nc.vector.tensor_scalar_max(z[:], ab4[:, :, 64], EPS)
                nc.vector.reciprocal(z[:], z[:])
                attn = wk2.tile([128, 256], BF16, name="attn", tag="attn")
                nc.vector.tensor_tensor(
                    out=attn[:].rearrange("p (h j) -> p h j", h=4),
                    in0=ab4[:, :, 0:64],
                    in1=z[:].unsqueeze(2).broadcast_to([128, 4, 64]),
                    op=mybir.AluOpType.mult)
                attnT = []
                for pr in range(2):
                    tp_ps = tp_ps_pool.tile([128, 128], BF16, name="tp_ps", tag="tp")
                    nc.tensor.transpose(tp_ps[:], attn[:, pr * 128:(pr + 1) * 128],
                                        ident[:])
                    aT = wk2.tile([128, 128], BF16, name="aT", tag="aT", bufs=6)
                    if pr == 0:
                        nc.vector.tensor_copy(aT[:], tp_ps[:])
                    else:
                        nc.scalar.activation(aT[:], tp_ps[:],
                                             mybir.ActivationFunctionType.Copy)
                    attnT.append(aT)

                out_sb = wk2.tile([128, E], BF16, name="out_sb", tag="out_sb", bufs=3)
                for nck in range(2):
                    op_ps = op_ps_pool.tile([128, 512], F32, name="op_ps", tag="op")
                    for pr in range(2):
                        nc.tensor.matmul(
                            op_ps[:],
                            attnT[pr][:],
                            wo3[:, pr, nck * 512:(nck + 1) * 512],
                            start=(pr == 0), stop=(pr == 1),
                        )
                    if nck == 0:
                        nc.scalar.activation(out_sb[:, 0:512], op_ps[:],
                                             mybir.ActivationFunctionType.Copy)
                    else:
                        nc.vector.tensor_copy(out_sb[:, 512:1024], op_ps[:])
                nc.sync.dma_start(out_d[lsl, :], out_sb[:])

    nc.compile()
    return nc


def host_prep(query, cos, sin, Wq, Wk, Wv, Wo, L=L_FULL, LT=32):
    """Build per-core input maps (fp16 compute payloads, bf16 out-proj)."""
    F16N = np.float16
    cos2 = np.ascontiguousarray(cos[0], dtype=np.float32)   # (L, D)
    sin2 = np.ascontiguousarray(sin[0], dtype=np.float32)
    cosT_rep = np.ascontiguousarray(np.tile(cos2.T, (2, 1))).astype(F16N)  # (128, L)
    sinT_rep = np.ascontiguousarray(np.tile(sin2.T, (2, 1))).astype(F16N)
    cosL = np.ascontiguousarray(
        cos2.reshape(LT, 128, D).transpose(1, 0, 2).reshape(128, LT * D)).astype(F16N)
    sinm2 = np.concatenate([-sin2[:, :D // 2], sin2[:, D // 2:]], axis=1)
    sinmL = np.ascontiguousarray(
        sinm2.reshape(LT, 128, D).transpose(1, 0, 2).reshape(128, LT * D)).astype(F16N)

    idx = (np.pi / 2) * np.arange(1, L + 1, dtype=np.float32) / L
    s_arr = np.sin(idx).astype(np.float32)
    c_arr = np.cos(idx).astype(np.float32)
    sc = np.empty((128, LT * 2), dtype=np.float32)
    sc[:, 0::2] = s_arr.reshape(LT, 128).T
    sc[:, 1::2] = c_arr.reshape(LT, 128).T
    sT_rep = np.ascontiguousarray(np.tile(s_arr[None, :], (128, 1))).astype(F16N)
    cT_rep = np.ascontiguousarray(np.tile(c_arr[None, :], (128, 1))).astype(F16N)

    p_rot = np.zeros((D, D), dtype=np.float32)
    for j in range(D // 2):
        p_rot[D // 2 + j, j] = -1.0   # rot[:, j] = -q[:, 32+j]
        p_rot[j, D // 2 + j] = 1.0    # rot[:, 32+j] = q[:, j]
    p2 = np.zeros((128, 128), dtype=np.float32)
    p2[0:64, 0:64] = p_rot
    p2[64:128, 64:128] = p_rot

    qbT = [np.ascontiguousarray(query[:, b, :].T).astype(F16N) for b in range(N_BATCH)]

    in_maps = []
    for c in range(N_CORES):
        b = c // 4
        r0 = (c % 4) * F_LOC
        in_maps.append({
            "qbT": qbT[b],
            "wqT": np.ascontiguousarray(Wq[r0:r0 + F_LOC, :].T).astype(F16N),
            "wkvT": np.ascontiguousarray(
                np.concatenate([Wk[r0:r0 + F_LOC, :].T, Wv[r0:r0 + F_LOC, :].T],
                               axis=1)).astype(F16N),
            "wo_rhs": np.ascontiguousarray(Wo[:, r0:r0 + F_LOC].T).astype(BF),
            "cosT_rep": cosT_rep,
            "sinT_rep": sinT_rep,
            "sT_rep": sT_rep,
            "cT_rep": cT_rep,
            "cosL": cosL,
            "sinmL": sinmL,
            "sc_t": sc,
            "sc_tr": sc.astype(F16N),
            "p2_rot": p2.astype(F16N),
            "ident_r": np.eye(128, dtype=np.float32).astype(BF),
        })
    return in_maps


_PROG_CACHE = {}


def run(inputs, trace=False, trace_kwargs=None):
    """Run on 8 NeuronCores; returns (output, BassKernelResults)."""
    from concourse.bass_utils import run_bass_kernel_spmd

    LT = L_FULL // 128
    if LT not in _PROG_CACHE:
        _PROG_CACHE[LT] = build_program(LT)
    nc = _PROG_CACHE[LT]
    in_maps = host_prep(**inputs)
    kw = {}
    if trace:
        kw["trace"] = True
        if trace_kwargs:
            kw.update(trace_kwargs)
    res = run_bass_kernel_spmd(nc, in_maps, core_ids=list(range(N_CORES)), **kw)
    partials = [res.results[c]["out_partial"] for c in range(N_CORES)]
    out = np.empty((L_FULL, N_BATCH, E), dtype=np.float32)
    for b in range(N_BATCH):
        acc = partials[4 * b].astype(np.float32)
        for c in range(4 * b + 1, 4 * b + 4):
            acc += partials[c].astype(np.float32)
        out[:, b, :] = acc
    return out, res


def kernel(**inputs):
    out, _ = run(inputs)
    return out
